# revision 6
# baseline (speedup 1.0000x reference)
"""Trainium2 Bass kernel for the quirky MultiHeadAttention module.

Reference computation (S = D = 4096, 16 "heads" that are chunks of 256 ROWS):
    q = x @ Wq.T + bq ; k = x @ Wk.T + bk ; v = x @ Wv.T + bv
    per head h (rows h*256..h*256+255):
        scores = split(v)_h @ split(k)_h.T / 64 ; attn = softmax(scores, -1)
        out_h  = attn @ split(q)_h
    result = concat(out_h) @ Wo.T + bo

Sharding: pure data-parallel over token rows. Each of the 8 cores owns 512
rows = exactly 2 complete "heads"; every stage (projections, attention,
output projection) is row-local given full weights, so no collectives.

Per-core dataflow (all matmuls in float32r = full-rate PE with fp32 storage):
  phase B/C: kT = (x@Wk.T+bk).T and vT likewise, tiled [feat 128, tok 512],
             spilled to DRAM scratch.
  phase A:   q natural [tok, feat], bias added via a K=1 ones-row matmul
             into the same PSUM accumulation group, spilled to DRAM.
  attention: S.T[j,i] = sum_d k[j,d] v[i,d] accumulated in PSUM,
             E.T = exp(S.T/64) (no max-subtraction needed: |logits| <~ 8),
             Z[i] = ones-column matmul over E.T partitions, zinv = 1/Z,
             broadcast zinv via ones-row matmul, normalize E.T in place,
             O.T[d,i] = sum_j q[j,d] * E.T_norm[j,i]  (kept in SBUF).
  final:     out rows = O.T.T @ Wo.T + bo, same structure as phase A.
"""

import numpy as np

import concourse.bass as bass
import concourse.bacc as bacc
import concourse.mybir as mybir
import concourse.tile as tile
from concourse.bass_utils import run_bass_kernel_spmd

F32 = mybir.dt.float32
F32R = mybir.dt.float32r
AF = mybir.ActivationFunctionType

D = 4096          # d_model == seq
NCORE = 8
SH = D // NCORE   # 512 token rows per core
KB = D // 128     # 32 contraction blocks of 128
NO = D // 512     # 8 output-feature chunks of 512
SM = SH // 128    # 4 token blocks of 128 per core
SCALE = 1.0 / 64.0  # 1/sqrt(4096)


def _build():
    nc = bacc.Bacc(
        "TRN2",
        target_bir_lowering=False,
        debug=False,
        enable_asserts=False,
        num_devices=NCORE,
    )

    xTp = nc.declare_dram_parameter("xTp", [128, KB, SH], F32, isOutput=False)
    wqp = nc.declare_dram_parameter("wqp", [NO, KB, 128, 512], F32, isOutput=False)
    wkp = nc.declare_dram_parameter("wkp", [KB, 128, KB, 128], F32, isOutput=False)
    wvp = nc.declare_dram_parameter("wvp", [KB, 128, KB, 128], F32, isOutput=False)
    wop = nc.declare_dram_parameter("wop", [NO, KB, 128, 512], F32, isOutput=False)
    bq_r = nc.declare_dram_parameter("bq_r", [1, D], F32, isOutput=False)
    bk_p = nc.declare_dram_parameter("bk_p", [128, KB], F32, isOutput=False)
    bv_p = nc.declare_dram_parameter("bv_p", [128, KB], F32, isOutput=False)
    bo_r = nc.declare_dram_parameter("bo_r", [1, D], F32, isOutput=False)
    ones_r = nc.declare_dram_parameter("ones_r", [1, 128], F32, isOutput=False)
    ones_c = nc.declare_dram_parameter("ones_c", [128, 1], F32, isOutput=False)
    out = nc.declare_dram_parameter("out", [SH, D], F32, isOutput=True)

    with tile.TileContext(nc) as tc:
        with (
            nc.allow_low_precision(reason="float32r tiles feeding PE matmuls"),
            tc.tile_pool(name="dram", bufs=1, space="DRAM") as dpool,
            tc.tile_pool(name="const", bufs=1) as cpool,
        ):
            q_dram = dpool.tile([SM, 128, NO, 512], F32, name="q_dram")
            kT_dram = dpool.tile([KB, 128, SH], F32, name="kT_dram")
            vT_dram = dpool.tile([KB, 128, SH], F32, name="vT_dram")

            ones_row = cpool.tile([1, 128], F32R, name="ones_row")
            nc.sync.dma_start(ones_row[:], ones_r[:].bitcast(F32R))
            ones_col = cpool.tile([128, 1], F32R, name="ones_col")
            nc.sync.dma_start(ones_col[:], ones_c[:].bitcast(F32R))
            zero_col = cpool.tile([128, 1], F32, name="zero_col")
            nc.vector.memset(zero_col[:], 0.0)
            bkv = cpool.tile([128, 2 * KB], F32, name="bkv")
            nc.sync.dma_start(bkv[:, 0:KB], bk_p[:])
            nc.sync.dma_start(bkv[:, KB : 2 * KB], bv_p[:])

            with tc.tile_pool(name="xpool", bufs=1) as xpool:
                xT = xpool.tile([128, KB, SH], F32R, name="xT")
                for kb in range(KB):
                    nc.sync.dma_start(xT[:, kb, :], xTp[:, kb, :].bitcast(F32R))

                # ---------------- phase B/C: kT and vT ----------------
                with (
                    tc.tile_pool(name="wslab", bufs=2) as wslab_pool,
                    tc.tile_pool(name="stbc", bufs=4) as stbc_pool,
                    tc.tile_pool(name="psbc", bufs=8, space="PSUM") as psbc_pool,
                ):
                    for which, (wp, dst) in enumerate(
                        ((wkp, kT_dram), (wvp, vT_dram))
                    ):
                        for m in range(KB):
                            slab = wslab_pool.tile(
                                [128, KB, 128], F32R, tag="slab",
                                name=f"slab_{which}_{m}",
                            )
                            nc.sync.dma_start(slab[:], wp[m][:].bitcast(F32R))
                            ps = psbc_pool.tile(
                                [128, SH], F32, tag="acc", name=f"pskv_{which}_{m}"
                            )
                            for kb in range(KB):
                                nc.tensor.matmul(
                                    ps[:],
                                    slab[:, kb, :],
                                    xT[:, kb, :],
                                    start=(kb == 0),
                                    stop=(kb == KB - 1),
                                )
                            st = stbc_pool.tile(
                                [128, SH], F32, tag="st", name=f"stkv_{which}_{m}"
                            )
                            nc.scalar.activation(
                                st[:], ps[:], AF.Identity,
                                bias=bkv[:, which * KB + m : which * KB + m + 1],
                            )
                            nc.sync.dma_start(dst[m][:], st[:])

                # ---------------- phase A: q natural ----------------
                with (
                    tc.tile_pool(name="wa", bufs=4) as wa_pool,
                    tc.tile_pool(name="sta", bufs=4) as sta_pool,
                    tc.tile_pool(name="ba", bufs=1) as ba_pool,
                    tc.tile_pool(name="psa", bufs=8, space="PSUM") as psa_pool,
                ):
                    bq_s = ba_pool.tile([1, D], F32R, name="bq_s")
                    nc.sync.dma_start(bq_s[:], bq_r[:].bitcast(F32R))
                    for n in range(NO):
                        pss = [
                            psa_pool.tile(
                                [128, 512], F32, tag="acc", name=f"psq_{n}_{m}"
                            )
                            for m in range(SM)
                        ]
                        for kb in range(KB):
                            wt = wa_pool.tile(
                                [128, 512], F32R, tag="wa", name=f"waq_{n}_{kb}"
                            )
                            nc.sync.dma_start(wt[:], wqp[n, kb][:].bitcast(F32R))
                            for m in range(SM):
                                nc.tensor.matmul(
                                    pss[m][:],
                                    xT[:, kb, m * 128 : (m + 1) * 128],
                                    wt[:],
                                    start=(kb == 0),
                                    stop=False,
                                )
                        for m in range(SM):
                            nc.tensor.matmul(
                                pss[m][:],
                                ones_row[:],
                                bq_s[0:1, n * 512 : (n + 1) * 512],
                                start=False,
                                stop=True,
                            )
                            st = sta_pool.tile(
                                [128, 512], F32, tag="sta", name=f"stq_{n}_{m}"
                            )
                            nc.vector.tensor_copy(st[:], pss[m][:])
                            nc.sync.dma_start(q_dram[m, :, n, :], st[:])

            # ---------------- attention + final projection ----------------
            with tc.tile_pool(name="otp", bufs=1) as otpool:
                OT = otpool.tile([128, KB, SH], F32R, name="OT")

                with (
                    tc.tile_pool(name="att", bufs=1) as apool,
                    tc.tile_pool(name="etp", bufs=2) as etpool,
                    tc.tile_pool(name="psS", bufs=2, space="PSUM") as psS_pool,
                    tc.tile_pool(name="psZ", bufs=1, space="PSUM") as psZ_pool,
                    tc.tile_pool(name="psB", bufs=1, space="PSUM") as psB_pool,
                    tc.tile_pool(name="psO", bufs=4, space="PSUM") as psO_pool,
                ):
                    for h in range(2):
                        psS = [
                            psS_pool.tile(
                                [128, 256], F32, tag="ps", name=f"psS_{h}_{jb}"
                            )
                            for jb in range(2)
                        ]
                        for kb in range(KB):
                            vt = apool.tile(
                                [128, 256], F32R, tag="vt", bufs=4,
                                name=f"vt_{h}_{kb}",
                            )
                            nc.sync.dma_start(
                                vt[:], vT_dram[kb, :, h * 256 : (h + 1) * 256].bitcast(F32R)
                            )
                            for jb in range(2):
                                kt = apool.tile(
                                    [128, 128], F32R, tag="kt", bufs=4,
                                    name=f"kt_{h}_{kb}_{jb}",
                                )
                                nc.sync.dma_start(
                                    kt[:],
                                    kT_dram[
                                        kb, :,
                                        h * 256 + jb * 128 : h * 256 + (jb + 1) * 128,
                                    ].bitcast(F32R),
                                )
                                nc.tensor.matmul(
                                    psS[jb][:],
                                    kt[:],
                                    vt[:],
                                    start=(kb == 0),
                                    stop=(kb == KB - 1),
                                )
                        ET = []
                        for jb in range(2):
                            et = etpool.tile(
                                [128, 256], F32R, tag="et", name=f"et_{h}_{jb}"
                            )
                            nc.scalar.activation(
                                et[:], psS[jb][:], AF.Exp,
                                bias=zero_col[:], scale=SCALE,
                            )
                            ET.append(et)
                        psz = psZ_pool.tile([1, 256], F32, tag="pz", name=f"psz_{h}")
                        for jb in range(2):
                            nc.tensor.matmul(
                                psz[:],
                                ones_col[:],
                                ET[jb][:],
                                start=(jb == 0),
                                stop=(jb == 1),
                            )
                        zinv = etpool.tile([1, 256], F32R, tag="zi", name=f"zinv_{h}")
                        nc.vector.reciprocal(zinv[:], psz[:])
                        pzb = psB_pool.tile([128, 256], F32, tag="pb", name=f"pzb_{h}")
                        nc.tensor.matmul(
                            pzb[:],
                            ones_row[:],
                            zinv[:],
                        )
                        for jb in range(2):
                            nc.vector.tensor_mul(ET[jb][:], ET[jb][:], pzb[:])
                        for db in range(KB):
                            pso = psO_pool.tile(
                                [128, 256], F32, tag="po", name=f"psO_{h}_{db}"
                            )
                            for jb in range(2):
                                qt = apool.tile(
                                    [128, 128], F32R, tag="qt", bufs=4,
                                    name=f"qt_{h}_{db}_{jb}",
                                )
                                nc.sync.dma_start(
                                    qt[:],
                                    q_dram[
                                        h * 2 + jb, :,
                                        db // 4,
                                        (db % 4) * 128 : (db % 4 + 1) * 128,
                                    ].bitcast(F32R),
                                )
                                nc.tensor.matmul(
                                    pso[:],
                                    qt[:],
                                    ET[jb][:],
                                    start=(jb == 0),
                                    stop=(jb == 1),
                                )
                            nc.vector.tensor_copy(
                                OT[:, db, h * 256 : (h + 1) * 256], pso[:]
                            )

                # ---------------- final: out = concat @ Wo.T + bo ----------------
                with (
                    tc.tile_pool(name="wf", bufs=4) as wf_pool,
                    tc.tile_pool(name="stf", bufs=4) as stf_pool,
                    tc.tile_pool(name="bf", bufs=1) as bf_pool,
                    tc.tile_pool(name="psf", bufs=8, space="PSUM") as psf_pool,
                ):
                    bo_s = bf_pool.tile([1, D], F32R, name="bo_s")
                    nc.sync.dma_start(bo_s[:], bo_r[:].bitcast(F32R))
                    for n in range(NO):
                        pss = [
                            psf_pool.tile(
                                [128, 512], F32, tag="acc", name=f"psf_{n}_{m}"
                            )
                            for m in range(SM)
                        ]
                        for kb in range(KB):
                            wt = wf_pool.tile(
                                [128, 512], F32R, tag="wf", name=f"wf_{n}_{kb}"
                            )
                            nc.sync.dma_start(wt[:], wop[n, kb][:].bitcast(F32R))
                            for m in range(SM):
                                nc.tensor.matmul(
                                    pss[m][:],
                                    OT[:, kb, m * 128 : (m + 1) * 128],
                                    wt[:],
                                    start=(kb == 0),
                                    stop=False,
                                )
                        for m in range(SM):
                            nc.tensor.matmul(
                                pss[m][:],
                                ones_row[:],
                                bo_s[0:1, n * 512 : (n + 1) * 512],
                                start=False,
                                stop=True,
                            )
                            st = stf_pool.tile(
                                [128, 512], F32, tag="stf", name=f"stf_{n}_{m}"
                            )
                            nc.vector.tensor_copy(st[:], pss[m][:])
                            nc.sync.dma_start(
                                out[m * 128 : (m + 1) * 128, n * 512 : (n + 1) * 512],
                                st[:],
                            )

    nc.compile()
    return nc


_NC_CACHE = None


def _pack_inputs(x, Wq, bq, Wk, bk, Wv, bv, Wo, bo):
    f = lambda a: np.ascontiguousarray(np.asarray(a, dtype=np.float32))
    x, Wq, bq, Wk, bk, Wv, bv, Wo, bo = map(
        f, (x, Wq, bq, Wk, bk, Wv, bv, Wo, bo)
    )
    WqT = np.ascontiguousarray(Wq.T)
    WoT = np.ascontiguousarray(Wo.T)
    shared = {
        "wqp": np.ascontiguousarray(
            WqT.reshape(KB, 128, NO, 512).transpose(2, 0, 1, 3)
        ),
        "wkp": np.ascontiguousarray(
            Wk.reshape(KB, 128, KB, 128).transpose(0, 3, 2, 1)
        ),
        "wvp": np.ascontiguousarray(
            Wv.reshape(KB, 128, KB, 128).transpose(0, 3, 2, 1)
        ),
        "wop": np.ascontiguousarray(
            WoT.reshape(KB, 128, NO, 512).transpose(2, 0, 1, 3)
        ),
        "bq_r": bq.reshape(1, D),
        "bk_p": np.ascontiguousarray(bk.reshape(KB, 128).T),
        "bv_p": np.ascontiguousarray(bv.reshape(KB, 128).T),
        "bo_r": bo.reshape(1, D),
        "ones_r": np.ones((1, 128), np.float32),
        "ones_c": np.ones((128, 1), np.float32),
    }
    in_maps = []
    for c in range(NCORE):
        xs = x[c * SH : (c + 1) * SH]
        xTp_c = np.ascontiguousarray(xs.T.reshape(KB, 128, SH).transpose(1, 0, 2))
        in_maps.append({"xTp": xTp_c, **shared})
    return in_maps


def run(inputs: dict, trace: bool = False, tmpdir=None):
    """Build (cached), run on 8 cores, return (full_output, BassKernelResults)."""
    global _NC_CACHE
    in_maps = _pack_inputs(**inputs)
    if _NC_CACHE is None:
        _NC_CACHE = _build()
    res = run_bass_kernel_spmd(
        _NC_CACHE, in_maps, list(range(NCORE)), trace=trace, tmpdir=tmpdir
    )
    full = np.concatenate(
        [res.results[c]["out"] for c in range(NCORE)], axis=0
    )
    return full, res


def kernel(x, Wq, bq, Wk, bk, Wv, bv, Wo, bo):
    full, _ = run(
        dict(x=x, Wq=Wq, bq=bq, Wk=Wk, bk=bk, Wv=Wv, bv=bv, Wo=Wo, bo=bo)
    )
    return full
